# revision 27
# baseline (speedup 1.0000x reference)
"""Trainium2 Bass kernel for a 2-layer GCN encoder (GCNConv x2 + global mean pool).

Math: with A' = A + I and deg = indegree(A') (symmetric-norm GCN),
    gcn(h, W, b) = D^-1/2 A' D^-1/2 (h) W + b
factorized as  out = dinv * (A' @ (dinv * h)) @ W + b   (dinv = deg^-1/2)
so the SpMM is pure 0/1 structure; per-edge norms become per-node row scales.

Sharding: dst-node ranges across 8 cores (6272 rows each). Each core:
  - bulk-gathers source rows of the (replicated) bf16 feature table from HBM
    with dma_gather (InstDMAGatherAnt) in <=1024-index instructions
    round-robined over the 4 SWDGE queues (4 descriptor rings drain
    concurrently -> ~3x one queue's throughput; the ring holds 1024
    descriptors, hence the cap). Indices are int16 relative to a mid-table
    base row: negative indices sign-extend into valid below-base addresses,
    so one stream covers the whole 50176-row table. Only TRAILING negatives
    are treated as skip-padding by the ucode, so the host guarantees every
    gather's final index is non-negative (swapping a >=base edge or pad
    into that slot within its dst tile, which is order-invariant).
  - streams each tile's self-loop block with one regular HWDGE DMA per block,
  - builds the 0/1 scatter matrix on VectorE in bf16 (compare dst-offset
    vs iota; the self-loop chunk's column is an identity pattern),
  - scatter-adds via TensorE one-hot bf16 matmuls accumulating in PSUM
    (psum_T is [din, dstoff], already transposed for the dense W matmul),
  - bf16 dense W matmul + rank-1 bias matmul + fused relu/dinv-scale.
Two SPMD launches (layer 1 -> host allgather of the 1.6MB/core slabs ->
layer 2 + graph pooling, accumulated across tiles in a single PSUM bank).
"""
import math
import numpy as np
import ml_dtypes

from concourse import bass, mybir, tile, bacc
from concourse.bass_utils import run_bass_kernel_spmd
from concourse._compat import get_trn_type

N_CORES = 8
P = 128          # partitions / tile rows
D = 128          # feature dim
G = 512          # number of graphs (fixed by the problem)
F32 = mybir.dt.float32
BF16 = mybir.dt.bfloat16
I16 = mybir.dt.int16

USE_BF16 = True     # bf16 gather table (half the random-gather bytes)
BASE = 25088        # gather base row: idx = src - BASE in [-25088, 25087]
BLK = 7             # dst tiles per gather block
GMAX = 1024         # max indices per dma_gather (SWDGE ring capacity)


def _wrap16(stream):
    """Wrap an index stream for dma_gather: entry j -> partition j%16,
    col j//16; replicated 8x down the 128 partitions (one copy per Q7 core)."""
    n = len(stream)
    assert n % 16 == 0
    block = np.asarray(stream, dtype=np.int16).reshape(n // 16, 16).T  # [16, n/16]
    return np.tile(block, (8, 1))                                      # [128, n/16]


# ---------------------------------------------------------------- host prep

def preprocess(x, edge_index, batch):
    N = x.shape[0]
    rows_per_core = int(math.ceil(N / (N_CORES * P))) * P
    npad = rows_per_core * N_CORES
    tiles_per_core = rows_per_core // P

    src = edge_index[0].astype(np.int64)
    dst = edge_index[1].astype(np.int64)
    # Self-loops are handled as a per-block sequential-DMA chunk (no gather
    # descriptors); only real edges go through the sorted gather stream.
    deg = (np.bincount(dst, minlength=N) + 1).astype(np.float32)
    dinv = 1.0 / np.sqrt(np.maximum(deg, 1.0))

    xhat = np.zeros((npad, D), dtype=np.float32)
    xhat[:N] = x.astype(np.float32) * dinv[:, None]

    order = np.argsort(dst)
    src_s = src[order]
    dst_s = dst[order]
    bounds = np.searchsorted(dst_s, np.arange(0, npad + 1, P))

    cnt = (bounds[1:] - bounds[:-1]).reshape(N_CORES, tiles_per_core)
    CA = np.maximum(1, (cnt.max(axis=0) + P - 1) // P).astype(np.int64)
    ca_max = int(CA.max()) + 1            # +1 for the self chunk in S/iota
    sum_ca = int(CA.sum())                # edge chunks (self excluded)

    nblk = (tiles_per_core + BLK - 1) // BLK
    blk_tiles = [list(range(b * BLK, min((b + 1) * BLK, tiles_per_core)))
                 for b in range(nblk)]
    Eb = [int(CA[ts].sum()) * P for ts in blk_tiles]   # idx per block
    g_max = max(Eb[b] // P + len(blk_tiles[b]) for b in range(nblk))

    idx_cols = sum(Eb) // 16
    idx_all = np.zeros((N_CORES, P, idx_cols), dtype=np.int16)
    # dstoff columns per tile: CA_t edge columns then the self column.
    dstoff = np.full((N_CORES, P, sum_ca + tiles_per_core), float(P),
                     dtype=ml_dtypes.bfloat16)

    for k in range(N_CORES):
        icol = 0
        cola = 0
        for b in range(nblk):
            stream = []
            boundary = []                  # tile segment bounds in the stream
            pos = 0
            for t in blk_tiles[b]:
                gt = k * tiles_per_core + t
                s, e = int(bounds[gt]), int(bounds[gt + 1])
                n = e - s
                Cp = int(CA[t]) * P
                buf = np.zeros(Cp, dtype=np.int64)    # pad idx = 0 (row BASE)
                buf[:n] = src_s[s:e] - BASE
                off = np.full(Cp, float(P), dtype=np.float32)
                off[:n] = (dst_s[s:e] - gt * P).astype(np.float32)
                stream.append(buf)
                boundary.append((pos, pos + Cp, n))
                dstoff[k, :, cola:cola + int(CA[t])] = (
                    off.reshape(int(CA[t]), P).T.astype(ml_dtypes.bfloat16))
                dstoff[k, :, cola + int(CA[t])] = (
                    np.arange(P, dtype=np.float32).astype(ml_dtypes.bfloat16))
                cola += int(CA[t]) + 1
                pos += Cp
            st = np.concatenate(stream)

            # every gather's final index must be non-negative (trailing
            # negatives are dropped by the ucode): swap within the dst tile.
            ends = sorted(set(list(range(GMAX - 1, Eb[b], GMAX)) + [Eb[b] - 1]))
            ends_set = set(ends)
            for pe in ends:
                if st[pe] >= 0:
                    continue
                ti = next(i for i, (a, z, n) in enumerate(boundary)
                          if a <= pe < z)
                a, z, n = boundary[ti]
                cand = [int(c) + a for c in np.nonzero(st[a:z] >= 0)[0]
                        if int(c) + a not in ends_set]
                assert cand, "tile with no swappable >=BASE edge nor pad"
                q = cand[0]
                st[pe], st[q] = st[q], st[pe]
                t0 = blk_tiles[b][ti]
                colt = int(CA[:t0].sum()) + t0  # dstoff col base of tile t0
                # swap dstoff entries: position -> (row, col) within tile
                r1, c1 = (pe - a) % P, (pe - a) // P
                r2, c2 = (q - a) % P, (q - a) // P
                v1 = dstoff[k, r1, colt + c1].copy()
                dstoff[k, r1, colt + c1] = dstoff[k, r2, colt + c2]
                dstoff[k, r2, colt + c2] = v1
            idx_all[k, :, icol:icol + Eb[b] // 16] = _wrap16(
                st.astype(np.int16))
            icol += Eb[b] // 16

    dinv_pad = np.zeros(npad, dtype=np.float32)
    dinv_pad[:N] = dinv
    dinv_slab = dinv_pad.reshape(N_CORES, tiles_per_core, P).transpose(0, 2, 1).copy()
    # sqrt(deg) per row (0 on pad rows): the bias is injected into PSUM as
    # sdeg[p]*b[j] so that the later *dinv row scale yields u*dinv + b exactly.
    sdeg_pad = np.zeros(npad, dtype=np.float32)
    sdeg_pad[:N] = np.sqrt(np.maximum(deg, 1.0))
    sdeg_rows = sdeg_pad.reshape(N_CORES, 1, tiles_per_core * P).astype(
        ml_dtypes.bfloat16)

    batch_pad = np.full(npad, -1, dtype=np.int64)
    batch_pad[:N] = batch.astype(np.int64)
    g0 = np.zeros(N_CORES, dtype=np.int64)
    batchoff = np.full((N_CORES, P, tiles_per_core), float(P),
                       dtype=ml_dtypes.bfloat16)
    for k in range(N_CORES):
        bb = batch_pad[k * rows_per_core:(k + 1) * rows_per_core]
        real = bb >= 0
        assert real.any()
        g0[k] = bb[real].min()
        span = int(bb[real].max() - g0[k]) + 1
        assert span <= P - 1, f"graph span {span} exceeds pooling tile"
        off = np.full(rows_per_core, float(P), dtype=np.float32)
        off[real] = (bb[real] - g0[k]).astype(np.float32)
        batchoff[k] = off.reshape(tiles_per_core, P).T.astype(ml_dtypes.bfloat16)

    iota = np.tile(np.arange(P, dtype=np.float32), (P, ca_max)).astype(
        ml_dtypes.bfloat16)

    cnt_g = np.bincount(batch.astype(np.int64), minlength=G).astype(np.float32)

    return dict(N=N, npad=npad, rows_per_core=rows_per_core,
                tiles_per_core=tiles_per_core,
                CA=CA, ca_max=ca_max, sum_ca=sum_ca,
                nblk=nblk, blk_tiles=blk_tiles, Eb=Eb, g_max=g_max,
                idx_cols=idx_cols, idx_all=idx_all, dstoff=dstoff,
                dinv_slab=dinv_slab, sdeg_rows=sdeg_rows,
                batchoff=batchoff, g0=g0,
                iota=iota, xhat=xhat, cnt_g=cnt_g)


# ---------------------------------------------------------------- device

def build_layer(pre, last_layer: bool, reps: int = 1, bf16_table: bool = True):
    """One SPMD program: blocked bulk gather + SpMM + dense matmul per tile.
    last_layer=False: out = dinv*relu(dinv * z)  -> hhat slab [rows_per_core, D]
    last_layer=True:  h2 = dinv * z, pooled[goff] += sum h2 -> pooled [P, D]
    """
    tiles = pre['tiles_per_core']
    CA = pre['CA']
    ca_max = pre['ca_max']
    sum_ca = pre['sum_ca']
    npad = pre['npad']
    nblk = pre['nblk']
    blk_tiles = pre['blk_tiles']
    Eb = pre['Eb']
    g_max = pre['g_max']
    ncols = sum_ca + tiles

    TDT = BF16 if bf16_table else F32
    nc = bacc.Bacc(get_trn_type() or "TRN2", target_bir_lowering=False, debug=False,
                   num_swdge_queues=4)
    table = nc.dram_tensor("table", [npad, D], TDT, kind="ExternalInput").ap()
    self_rows = nc.dram_tensor("self_rows", [tiles * P, D], TDT,
                               kind="ExternalInput").ap()
    idx_all = nc.dram_tensor("idx_all", [P, pre['idx_cols']], I16,
                             kind="ExternalInput").ap()
    dstoff = nc.dram_tensor("dstoff", [P, ncols], BF16, kind="ExternalInput").ap()
    iota = nc.dram_tensor("iota", [P, ca_max * P], BF16, kind="ExternalInput").ap()
    Wt = nc.dram_tensor("W", [D, D], BF16, kind="ExternalInput").ap()
    bt = nc.dram_tensor("b", [1, D], BF16, kind="ExternalInput").ap()
    dinv = nc.dram_tensor("dinv", [P, tiles], F32, kind="ExternalInput").ap()
    sdeg = nc.dram_tensor("sdeg", [1, tiles * P], BF16, kind="ExternalInput").ap()
    if last_layer:
        batchoff = nc.dram_tensor("batchoff", [P, tiles], BF16,
                                  kind="ExternalInput").ap()
        pooled = nc.dram_tensor("pooled", [P, D], F32, kind="ExternalOutput").ap()
    else:
        hhat = nc.dram_tensor("hhat", [tiles * P, D], TDT, kind="ExternalOutput").ap()

    with tile.TileContext(nc) as tc:
        with tc.tile_pool(name="const", bufs=1) as cp, \
             tc.tile_pool(name="gather", bufs=3) as gp, \
             tc.tile_pool(name="sel", bufs=4) as sp, \
             tc.tile_pool(name="small", bufs=3) as mp, \
             tc.tile_pool(name="hh", bufs=2) as hp, \
             tc.tile_pool(name="ps1", bufs=2, space="PSUM") as pp1, \
             tc.tile_pool(name="ps2", bufs=2, space="PSUM") as pp2:
            ix_t = cp.tile([P, pre['idx_cols']], I16)
            dst_t = cp.tile([P, ncols], BF16)
            iota_t = cp.tile([P, ca_max * P], BF16)
            W_t = cp.tile([D, D], BF16)
            b_t = cp.tile([1, D], BF16)
            sdeg_t = cp.tile([1, tiles * P], BF16)
            dinv_t = cp.tile([P, tiles], F32)
            nc.sync.dma_start(out=ix_t[:], in_=idx_all[:])
            nc.sync.dma_start(out=dst_t[:], in_=dstoff[:])
            nc.sync.dma_start(out=iota_t[:], in_=iota[:])
            nc.sync.dma_start(out=W_t[:], in_=Wt[:])
            nc.sync.dma_start(out=b_t[:], in_=bt[:])
            nc.sync.dma_start(out=sdeg_t[:], in_=sdeg[:])
            nc.sync.dma_start(out=dinv_t[:], in_=dinv[:])
            if last_layer:
                boff_t = cp.tile([P, tiles], BF16)
                nc.sync.dma_start(out=boff_t[:], in_=batchoff[:])
                pooled_sb = cp.tile([P, D], F32)

            for rep in range(reps):
                icol = 0
                cola = 0
                gq = 0                       # global SWDGE queue round-robin
                if last_layer:
                    pool_ps = pp2.tile([P, D], F32, space="PSUM", tag="pool_ps")
                else:
                    # whole layer output staged in SBUF (12.25KB/partition);
                    # one end-of-rep write keeps HBM free of read/write
                    # turnarounds while the latency-bound gathers run.
                    hh_all = hp.tile([P, tiles * D], TDT, tag="hha")
                for b in range(nblk):
                    ts = blk_tiles[b]
                    nb = len(ts)
                    ech = Eb[b] // P          # edge chunks in block
                    g = gp.tile([P, g_max * D], TDT, tag="g")
                    # split the block stream into <=1024-idx gathers (ring
                    # capacity) round-robined over the 4 SWDGE queues.
                    done = 0
                    while done < Eb[b]:
                        n = min(GMAX, Eb[b] - done)
                        nc.gpsimd.dma_gather(
                            g[:, (done // P) * D:((done + n) // P) * D]
                            .rearrange("p (c j) -> p c j", j=D),
                            table[BASE:, :],
                            ix_t[:, icol + done // 16:icol + (done + n) // 16],
                            n, n, D, queue_num=gq % 4)
                        gq += 1
                        done += n
                    # self-loop chunks: contiguous rows via the ACT HWDGE
                    # ring (nc.scalar) — keeping them off the SP ring where
                    # the compute-dependent hhat writes would queue ahead of
                    # them in FIFO order and stall the next block.
                    nc.scalar.dma_start(
                        out=g[:, ech * D:(ech + nb) * D]
                        .rearrange("p (c j) -> p c j", j=D),
                        in_=self_rows[ts[0] * P:(ts[0] + nb) * P, :]
                        .rearrange("(c p) j -> p c j", p=P))
                    icol += Eb[b] // 16

                    eoff = 0
                    for i, t in enumerate(ts):
                        ce = int(CA[t])
                        cat = ce + 1
                        S = sp.tile([P, ca_max * D], BF16, tag="s")
                        nc.vector.tensor_tensor(
                            out=S[:, :cat * D].rearrange("p (c j) -> p c j", j=D),
                            in0=dst_t[:, cola:cola + cat].to_broadcast([P, cat, D]),
                            in1=iota_t[:, :cat * D].rearrange("p (c j) -> p c j", j=D),
                            op=mybir.AluOpType.is_equal)
                        # chunk order in g: [block edge chunks][block self]
                        gsrc = ([(eoff + c) * D for c in range(ce)]
                                + [(ech + i) * D])
                        psumT = pp1.tile([P, D], F32, space="PSUM", tag="pT")
                        for c in range(cat):
                            nc.tensor.matmul(out=psumT[:],
                                             lhsT=g[:, gsrc[c]:gsrc[c] + D],
                                             rhs=S[:, c * D:(c + 1) * D],
                                             start=(c == 0), stop=(c == cat - 1))
                        lhs_sb = mp.tile([P, D], BF16, tag="lhs")
                        nc.vector.tensor_copy(out=lhs_sb[:], in_=psumT[:])
                        psum2 = pp2.tile([P, D], F32, space="PSUM", tag="p2")
                        nc.tensor.matmul(out=psum2[:], lhsT=lhs_sb[:], rhs=W_t[:],
                                         start=True, stop=False)
                        nc.tensor.matmul(out=psum2[:],
                                         lhsT=sdeg_t[:, t * P:(t + 1) * P],
                                         rhs=b_t[:], start=False, stop=True)
                        out_sb = mp.tile([P, D], F32, tag="out")
                        if last_layer:
                            nc.scalar.activation(
                                out=out_sb[:], in_=psum2[:],
                                func=mybir.ActivationFunctionType.Copy,
                                scale=dinv_t[:, t:t + 1])
                            Pt = sp.tile([P, D], F32, tag="pool_sel")
                            nc.vector.tensor_tensor(
                                out=Pt[:],
                                in0=boff_t[:, t:t + 1].to_broadcast([P, D]),
                                in1=iota_t[:, :D],
                                op=mybir.AluOpType.is_equal)
                            # graph pooling accumulates across all tiles in
                            # one PSUM bank (one accumulation group per rep)
                            nc.tensor.matmul(out=pool_ps[:], lhsT=Pt[:],
                                             rhs=out_sb[:],
                                             start=(t == 0),
                                             stop=(t == tiles - 1))
                        else:
                            nc.scalar.activation(
                                out=out_sb[:], in_=psum2[:],
                                func=mybir.ActivationFunctionType.Relu,
                                scale=dinv_t[:, t:t + 1])
                            nc.vector.tensor_scalar_mul(
                                out=hh_all[:, t * D:(t + 1) * D], in0=out_sb[:],
                                scalar1=dinv_t[:, t:t + 1])
                        eoff += ce
                        cola += cat
                if not last_layer:
                    nc.sync.dma_start(
                        out=hhat[:].rearrange("(c p) j -> p c j", p=P),
                        in_=hh_all[:].rearrange("p (c j) -> p c j", j=D))
                if last_layer:
                    nc.vector.tensor_copy(out=pooled_sb[:], in_=pool_ps[:])
                    nc.sync.dma_start(out=pooled[:], in_=pooled_sb[:])
    nc.compile()
    return nc


def _in_maps(pre, table_np, W, b, last_layer):
    maps = []
    rpc = pre['rows_per_core']
    for k in range(N_CORES):
        m = dict(table=table_np,
                 self_rows=np.ascontiguousarray(table_np[k * rpc:(k + 1) * rpc]),
                 idx_all=pre['idx_all'][k],
                 dstoff=pre['dstoff'][k],
                 iota=pre['iota'],
                 W=np.ascontiguousarray(W).astype(ml_dtypes.bfloat16),
                 b=np.ascontiguousarray(b).astype(ml_dtypes.bfloat16).reshape(1, D),
                 dinv=pre['dinv_slab'][k],
                 sdeg=pre['sdeg_rows'][k])
        if last_layer:
            m['batchoff'] = pre['batchoff'][k]
        maps.append(m)
    return maps


def kernel(x, edge_index, batch, W1, b1, W2, b2):
    x = np.asarray(x); edge_index = np.asarray(edge_index)
    batch = np.asarray(batch)
    W1 = np.asarray(W1); b1 = np.asarray(b1)
    W2 = np.asarray(W2); b2 = np.asarray(b2)

    pre = preprocess(x, edge_index, batch)
    core_ids = list(range(N_CORES))

    tdt = ml_dtypes.bfloat16 if USE_BF16 else np.float32
    table1 = pre['xhat'].astype(tdt)
    nc1 = build_layer(pre, last_layer=False, bf16_table=USE_BF16)
    res1 = run_bass_kernel_spmd(nc1, _in_maps(pre, table1, W1, b1, False),
                                core_ids).results

    h1hat = np.zeros((pre['npad'], D), dtype=tdt)
    rpc = pre['rows_per_core']
    for k in range(N_CORES):
        h1hat[k * rpc:(k + 1) * rpc] = res1[k]['hhat']

    nc2 = build_layer(pre, last_layer=True, bf16_table=USE_BF16)
    res2 = run_bass_kernel_spmd(nc2, _in_maps(pre, h1hat, W2, b2, True),
                                core_ids).results

    pooled = np.zeros((G, D), dtype=np.float32)
    for k in range(N_CORES):
        part = res2[k]['pooled']
        g0 = int(pre['g0'][k])
        span = min(P, G - g0)
        pooled[g0:g0 + span] += part[:span]
    return pooled / np.maximum(pre['cnt_g'], 1.0)[:, None]
